# revision 2
# baseline (speedup 1.0000x reference)
"""MultiHeadAttention on 8 TRN2 NeuronCores.

Head-tensor-parallel: core c owns heads 2c, 2c+1 (feature cols 128c:128c+128
of the QKV projections, rows 128c:128c+128 of wo). Each core computes its
partial output [4096, 1024]; host sums the 8 partials (row-parallel O proj).

Math per core (fp32r matmuls, f32 psum):
  Q^T/K^T/V^T [128, 4096] = Wc^T @ x^T (+bias via ACT)
  per (batch, head): S^T chunk = K^T_h' @ Q^T_h -> exp(0.125*S) on ACT
  O^T|colsum = (V|1)^T-style: lhsT = V natural chunk [128, 65], accumulate
  normalize via DVE reciprocal + ones-row broadcast matmul
  out partial = O_a^T.T @ wo_a + O_b^T.T @ wo_b + bo (core 0 only)
"""

import numpy as np

import concourse.bass as bass
import concourse.tile as tile
from concourse import bacc, mybir
from concourse.bass_utils import run_bass_kernel_spmd

N_CORES = 8
B, S, D = 2, 2048, 1024
TOK = B * S  # 4096
DK = 64
F32 = mybir.dt.float32
F32R = mybir.dt.float32r
Exp = mybir.ActivationFunctionType.Exp
Identity = mybir.ActivationFunctionType.Identity

_cache = {}


def _build(repeat=1):
    nc = bacc.Bacc("TRN2", target_bir_lowering=False, debug=False,
                   num_devices=N_CORES)
    xT_d = nc.dram_tensor("xT", [D, TOK], F32, kind="ExternalInput").ap()
    wq_d = nc.dram_tensor("wq", [128, D], F32, kind="ExternalInput").ap()
    wk_d = nc.dram_tensor("wk", [128, D], F32, kind="ExternalInput").ap()
    wv_d = nc.dram_tensor("wv", [128, D], F32, kind="ExternalInput").ap()
    bq_d = nc.dram_tensor("bq", [128, 1], F32, kind="ExternalInput").ap()
    bk_d = nc.dram_tensor("bk", [128, 1], F32, kind="ExternalInput").ap()
    bv_d = nc.dram_tensor("bv", [128, 1], F32, kind="ExternalInput").ap()
    woa_d = nc.dram_tensor("woa", [64, D], F32, kind="ExternalInput").ap()
    wob_d = nc.dram_tensor("wob", [64, D], F32, kind="ExternalInput").ap()
    bo_d = nc.dram_tensor("bo", [128, D], F32, kind="ExternalInput").ap()
    id_d = nc.dram_tensor("iden", [128, 128], F32, kind="ExternalInput").ap()
    vones_d = nc.dram_tensor("vones", [128, 16, 1], F32, kind="ExternalInput").ap()
    out_d = nc.dram_tensor("out", [TOK, D], F32, kind="ExternalOutput").ap()

    with tile.TileContext(nc) as tc:
        with tc.tile_pool(name="persist", bufs=1) as pp:
            wq_sb = pp.tile([128, D], F32R, tag="wq")
            wk_sb = pp.tile([128, D], F32R, tag="wk")
            wv_sb = pp.tile([128, D], F32R, tag="wv")
            nc.gpsimd.dma_start(wq_sb[:], wq_d[:].bitcast(F32R))
            nc.gpsimd.dma_start(wk_sb[:], wk_d[:].bitcast(F32R))
            nc.gpsimd.dma_start(wv_sb[:], wv_d[:].bitcast(F32R))
            woa_sb = pp.tile([64, D], F32R, tag="woa")
            wob_sb = pp.tile([64, D], F32R, tag="wob")
            nc.gpsimd.dma_start(woa_sb[:], woa_d[:].bitcast(F32R))
            nc.gpsimd.dma_start(wob_sb[:], wob_d[:].bitcast(F32R))
            bq_sb = pp.tile([128, 1], F32, tag="bq")
            bk_sb = pp.tile([128, 1], F32, tag="bk")
            bv_sb = pp.tile([128, 1], F32, tag="bv")
            nc.gpsimd.dma_start(bq_sb[:], bq_d[:])
            nc.gpsimd.dma_start(bk_sb[:], bk_d[:])
            nc.gpsimd.dma_start(bv_sb[:], bv_d[:])
            bo_sb = pp.tile([128, D], F32, tag="bo")
            nc.gpsimd.dma_start(bo_sb[:], bo_d[:])
            id_sb = pp.tile([128, 128], F32R, tag="iden")
            nc.gpsimd.dma_start(id_sb[:], id_d[:].bitcast(F32R))

            QT = pp.tile([128, TOK], F32R, tag="QT")
            KT = pp.tile([128, TOK], F32R, tag="KT")
            VT = pp.tile([128, TOK], F32R, tag="VT")
            Vn = [pp.tile([128, 16, 65], F32R, tag=f"vn{p}", name=f"vn{p}")
                  for p in range(4)]
            for p in range(4):
                nc.gpsimd.dma_start(Vn[p][:, :, 64:65], vones_d[:].bitcast(F32R))
            OTa = pp.tile([64, TOK], F32R, tag="ota")
            OTb = pp.tile([64, TOK], F32R, tag="otb")

            ones_f = pp.tile([128, 64], F32, tag="onesf")
            nc.vector.memset(ones_f[:], 1.0)
            onesr = pp.tile([128, 64], F32R, tag="onesr")
            nc.vector.tensor_copy(onesr[:], ones_f[:])

            for _ in range(repeat):
                _body(nc, tc, xT_d, out_d,
                      (wq_sb, wk_sb, wv_sb), (bq_sb, bk_sb, bv_sb),
                      (woa_sb, wob_sb), bo_sb, id_sb, onesr,
                      QT, KT, VT, Vn, OTa, OTb)
    nc.compile()
    return nc


def _body(nc, tc, xT_d, out_d, w_sbs, b_sbs, wo_sbs, bo_sb, id_sb, onesr,
          QT, KT, VT, Vn, OTa, OTb):
    PSUM = bass.MemorySpace.PSUM
    dsts = (QT, KT, VT)

    # ---- Phase 1: QKV projections + V transpose to natural layout ----
    with (
        tc.tile_pool(name="xt", bufs=2) as xtp,
        tc.tile_pool(name="p1psum", bufs=1, space=PSUM) as p1p,
        tc.tile_pool(name="tpsum", bufs=2, space=PSUM) as tpp,
        tc.tile_pool(name="vstage", bufs=2) as vsp,
    ):
        for g in range(4):  # token groups of 1024
            accs = [[p1p.tile([128, 512], F32, tag=f"acc{w}{t}",
                              name=f"acc{w}{t}")
                     for t in range(2)] for w in range(3)]
            for j in range(8):  # contraction chunks of 128
                xj = xtp.tile([128, 1024], F32R, tag="xj", name="xj")
                nc.gpsimd.dma_start(
                    xj[:], xT_d[128 * j:128 * (j + 1),
                                1024 * g:1024 * (g + 1)].bitcast(F32R))
                for w in range(3):
                    for t in range(2):
                        nc.tensor.matmul(
                            accs[w][t][:],
                            w_sbs[w][:, 128 * j:128 * (j + 1)],
                            xj[:, 512 * t:512 * (t + 1)],
                            start=(j == 0), stop=(j == 7))
            for w in range(3):
                for t in range(2):
                    col = 1024 * g + 512 * t
                    nc.scalar.activation(
                        dsts[w][:, col:col + 512], accs[w][t][:],
                        Identity, bias=b_sbs[w][:], scale=1.0)
            # V natural for this group's 8 key chunks x 2 heads
            b_idx = g // 2
            for hh in range(2):
                pr = 2 * b_idx + hh
                for kc in range(8):
                    kl = (g % 2) * 8 + kc
                    col = 1024 * g + 128 * kc
                    tps = tpp.tile([128, 64], F32R, tag="tps", name="tps")
                    nc.tensor.transpose(
                        tps[:],
                        VT[64 * hh:64 * (hh + 1), col:col + 128],
                        id_sb[64 * hh:64 * (hh + 1), 64 * hh:64 * (hh + 1)])
                    nc.vector.tensor_copy(Vn[pr][:, kl, 0:64], tps[:])

    # ---- Phase 2: attention per (batch, head) pair ----
    with (
        tc.tile_pool(name="pt", bufs=3) as ptp,
        tc.tile_pool(name="spsum", bufs=2, space=PSUM) as sp,
        tc.tile_pool(name="opsum", bufs=2, space=PSUM) as op,
        tc.tile_pool(name="bpsum", bufs=2, space=PSUM) as bp,
        tc.tile_pool(name="nrm", bufs=2) as nrm,
    ):
        for p in range(4):
            b_idx, hh = divmod(p, 2)
            base = 2048 * b_idx
            KT_h = KT[64 * hh:64 * (hh + 1), :]
            QT_h = QT[64 * hh:64 * (hh + 1), :]
            OT_h = OTa if hh == 0 else OTb
            for qt in range(4):
                q0 = base + 512 * qt
                o_acc = op.tile([65, 512], F32, tag="oacc", name="o_acc")
                for kc in range(16):
                    k0 = base + 128 * kc
                    s_ps = sp.tile([128, 512], F32, tag="sps", name="s_ps")
                    nc.tensor.matmul(s_ps[:], KT_h[:, k0:k0 + 128],
                                     QT_h[:, q0:q0 + 512],
                                     start=True, stop=True)
                    pt_t = ptp.tile([128, 512], F32R, tag="pt", name="pt_t")
                    nc.scalar.activation(pt_t[:], s_ps[:], Exp,
                                         bias=0.0, scale=0.125)
                    nc.tensor.matmul(o_acc[:], Vn[p][:, kc, :], pt_t[:],
                                     start=(kc == 0), stop=(kc == 15))
                r_f = nrm.tile([128, 512], F32, tag="rf", name="r_f")
                nc.vector.reciprocal(r_f[64:65, :], o_acc[64:65, :])
                r_t = nrm.tile([128, 512], F32R, tag="rt", name="r_t")
                nc.vector.tensor_copy(r_t[64:65, :], r_f[64:65, :])
                bc_ps = bp.tile([64, 512], F32, tag="bc", name="bc_ps")
                nc.tensor.matmul(bc_ps[:], onesr[64:65, :], r_t[64:65, :],
                                 start=True, stop=True)
                bc_sb = nrm.tile([64, 512], F32, tag="bcs", name="bc_sb")
                nc.vector.tensor_copy(bc_sb[:], bc_ps[:])
                nc.vector.tensor_mul(OT_h[:, q0:q0 + 512], o_acc[0:64, :],
                                     bc_sb[:])

    # ---- Phase 3: output projection + bias + DMA out ----
    with (
        tc.tile_pool(name="fout", bufs=2) as fo,
        tc.tile_pool(name="fpsum", bufs=2, space=PSUM) as fp,
    ):
        woa_sb, wob_sb = wo_sbs
        for tt in range(32):
            t0 = 128 * tt
            acc = fp.tile([128, 1024], F32, tag="facc", name="acc")
            for ns in range(2):
                n0 = 512 * ns
                nc.tensor.matmul(acc[:, n0:n0 + 512], OTa[:, t0:t0 + 128],
                                 woa_sb[:, n0:n0 + 512],
                                 start=True, stop=False)
                nc.tensor.matmul(acc[:, n0:n0 + 512], OTb[:, t0:t0 + 128],
                                 wob_sb[:, n0:n0 + 512],
                                 start=False, stop=True)
            o_sb = fo.tile([128, 1024], F32, tag="fo", name="o_sb")
            nc.vector.tensor_add(o_sb[:], acc[:], bo_sb[:])
            nc.gpsimd.dma_start(out_d[t0:t0 + 128, :], o_sb[:])


def _in_maps(x, wq, bq, wk, bk, wv, bv, wo, bo):
    xT = np.ascontiguousarray(x.reshape(TOK, D).T)

    def arr_w(w, c):
        wc = w[:, 128 * c:128 * (c + 1)]
        return np.ascontiguousarray(
            wc.reshape(8, 128, 128).transpose(1, 0, 2).reshape(128, D))

    iden = np.eye(128, dtype=np.float32)
    vones = np.ones((128, 16, 1), dtype=np.float32)
    bo_bc = np.broadcast_to(bo, (128, D)).astype(np.float32)
    bo_zero = np.zeros((128, D), dtype=np.float32)
    maps = []
    for c in range(N_CORES):
        sl = slice(128 * c, 128 * (c + 1))
        maps.append({
            "xT": xT,
            "wq": arr_w(wq, c), "wk": arr_w(wk, c), "wv": arr_w(wv, c),
            "bq": np.ascontiguousarray(bq[sl].reshape(128, 1)),
            "bk": np.ascontiguousarray(bk[sl].reshape(128, 1)),
            "bv": np.ascontiguousarray(bv[sl].reshape(128, 1)),
            "woa": np.ascontiguousarray(wo[128 * c:128 * c + 64, :]),
            "wob": np.ascontiguousarray(wo[128 * c + 64:128 * (c + 1), :]),
            "bo": bo_bc if c == 0 else bo_zero,
            "iden": iden,
            "vones": vones,
        })
    return maps


def kernel(**inputs):
    x = np.asarray(inputs["x"], dtype=np.float32)
    maps = _in_maps(
        x,
        np.asarray(inputs["wq"], np.float32), np.asarray(inputs["bq"], np.float32),
        np.asarray(inputs["wk"], np.float32), np.asarray(inputs["bk"], np.float32),
        np.asarray(inputs["wv"], np.float32), np.asarray(inputs["bv"], np.float32),
        np.asarray(inputs["wo"], np.float32), np.asarray(inputs["bo"], np.float32),
    )
    if "nc" not in _cache:
        _cache["nc"] = _build()
    res = run_bass_kernel_spmd(_cache["nc"], maps,
                               core_ids=list(range(N_CORES)), trace=False)
    total = res.results[0]["out"].astype(np.float32)
    for c in range(1, N_CORES):
        total += res.results[c]["out"]
    return total.reshape(B, S, D)


# revision 3
# speedup vs baseline: 1.0902x; 1.0902x over previous
"""MultiHeadAttention on 8 TRN2 NeuronCores — v2.

Head-tensor-parallel with on-device collectives:
- Each core uploads only its 512-token packed x slice (bf16) + its
  head-slice weights (bf16). AllGather assembles full x on every core.
- Phase 1: Q/K projections [dk, tok] via 8-chunk chains with the bias
  folded in as a K=1 outer-product chain element; V computed natively
  [tok, dk] (x chunk stationary) so no transposes are needed.
- Phase 2: per (batch, head): S chunks in fp32r, two 512-wide key
  blocks packed per [128,1024] psum tile so one exp ACT covers both;
  (V|1) chains accumulate O^T plus softmax denominators; normalize via
  reciprocal + ones-row broadcast matmul.
- Phase 3: partial out = OT^T @ wo_slice (+bo on core 0 only, K=1
  chain element); ReduceScatter sums partials and hands each core its
  512-token output slice.
"""

import numpy as np
import ml_dtypes

import concourse.bass as bass
import concourse.tile as tile
from concourse import bacc, mybir
from concourse.bass_utils import run_bass_kernel_spmd

N_CORES = 8
B, S, D = 2, 2048, 1024
TOK = B * S  # 4096
F32 = mybir.dt.float32
F32R = mybir.dt.float32r
BF16 = mybir.dt.bfloat16
Exp = mybir.ActivationFunctionType.Exp
Identity = mybir.ActivationFunctionType.Identity
BF = ml_dtypes.bfloat16

_cache = {}


def _build(repeat=1):
    nc = bacc.Bacc("TRN2", target_bir_lowering=False, debug=False,
                   num_devices=N_CORES)
    xp_d = nc.dram_tensor("xp", [128, 8, 512], BF16, kind="ExternalInput").ap()
    wq_d = nc.dram_tensor("wqp", [128, 8, 128], BF16, kind="ExternalInput").ap()
    wk_d = nc.dram_tensor("wkp", [128, 8, 128], BF16, kind="ExternalInput").ap()
    wv_d = nc.dram_tensor("wvp", [128, 8, 128], BF16, kind="ExternalInput").ap()
    wo_d = nc.dram_tensor("wos", [128, D], BF16, kind="ExternalInput").ap()
    bq_d = nc.dram_tensor("bqr", [1, 128], BF16, kind="ExternalInput").ap()
    bk_d = nc.dram_tensor("bkr", [1, 128], BF16, kind="ExternalInput").ap()
    bv_d = nc.dram_tensor("bvr", [1, 128], BF16, kind="ExternalInput").ap()
    bo_d = nc.dram_tensor("bor", [1, D], BF16, kind="ExternalInput").ap()
    on_d = nc.dram_tensor("onesb", [1, 512], BF16, kind="ExternalInput").ap()
    vo_d = nc.dram_tensor("vones", [128, 16, 1], F32, kind="ExternalInput").ap()
    out_d = nc.dram_tensor("out", [512, D], F32, kind="ExternalOutput").ap()

    with tile.TileContext(nc) as tc:
        with (
            tc.tile_pool(name="dram", bufs=1, space="DRAM") as dram,
            tc.tile_pool(name="persist", bufs=1) as pp,
        ):
            xb_bo = dram.tile([128, 8, 512], BF16, tag="xbo")
            gath_x = dram.tile([1024, 8, 512], BF16, tag="gx")
            part_d = dram.tile([TOK, D], F32, tag="part")
            rs_d = dram.tile([512, D], F32, tag="rsd")

            wq_sb = pp.tile([128, 8, 128], BF16, tag="wq")
            wk_sb = pp.tile([128, 8, 128], BF16, tag="wk")
            wv_sb = pp.tile([128, 8, 128], BF16, tag="wv")
            wo_sb = pp.tile([128, D], BF16, tag="wo")
            nc.gpsimd.dma_start(wq_sb[:], wq_d[:])
            nc.gpsimd.dma_start(wk_sb[:], wk_d[:])
            nc.gpsimd.dma_start(wv_sb[:], wv_d[:])
            nc.gpsimd.dma_start(wo_sb[:], wo_d[:])
            bq_sb = pp.tile([1, 128], BF16, tag="bq")
            bk_sb = pp.tile([1, 128], BF16, tag="bk")
            bv_sb = pp.tile([1, 128], BF16, tag="bv")
            bo_sb = pp.tile([1, D], BF16, tag="bo")
            ones_sb = pp.tile([1, 512], BF16, tag="ones")
            nc.gpsimd.dma_start(bq_sb[:], bq_d[:])
            nc.gpsimd.dma_start(bk_sb[:], bk_d[:])
            nc.gpsimd.dma_start(bv_sb[:], bv_d[:])
            nc.gpsimd.dma_start(bo_sb[:], bo_d[:])
            nc.gpsimd.dma_start(ones_sb[:], on_d[:])

            QT = pp.tile([128, TOK], F32R, tag="QT")
            KT = pp.tile([128, TOK], F32R, tag="KT")
            Vn = [pp.tile([128, 16, 65], F32R, tag=f"vn{p}", name=f"vn{p}")
                  for p in range(4)]
            for p in range(4):
                nc.gpsimd.dma_start(Vn[p][:, :, 64:65], vo_d[:].bitcast(F32R))
            OTb = pp.tile([128, TOK], BF16, tag="otb")

            ones_f = pp.tile([128, 64], F32, tag="onesf")
            nc.vector.memset(ones_f[:], 1.0)
            onesr = pp.tile([128, 64], F32R, tag="onesr")
            nc.vector.tensor_copy(onesr[:], ones_f[:])

            for _ in range(repeat):
                _body(nc, tc, xp_d, out_d, xb_bo, gath_x, part_d, rs_d,
                      (wq_sb, wk_sb, wv_sb), (bq_sb, bk_sb, bv_sb),
                      wo_sb, bo_sb, ones_sb, onesr, QT, KT, Vn, OTb)
    nc.compile()
    return nc


def _body(nc, tc, xp_d, out_d, xb_bo, gath_x, part_d, rs_d,
          w_sbs, b_sbs, wo_sb, bo_sb, ones_sb, onesr, QT, KT, Vn, OTb):
    PSUM = bass.MemorySpace.PSUM
    groups = [list(range(N_CORES))]
    wq_sb, wk_sb, wv_sb = w_sbs
    bq_sb, bk_sb, bv_sb = b_sbs

    # ---- AllGather x ----
    nc.gpsimd.dma_start(xb_bo[:], xp_d[:])
    nc.gpsimd.collective_compute(
        "AllGather", mybir.AluOpType.bypass, replica_groups=groups,
        ins=[xb_bo[:].opt()], outs=[gath_x[:].opt()])

    # ---- Phase 1: Q/K [dk, tok] + V natural [tok, dk] ----
    with (
        tc.tile_pool(name="xt", bufs=2) as xtp,
        tc.tile_pool(name="qkpsum", bufs=2, space=PSUM) as qkp,
        tc.tile_pool(name="vpsum", bufs=4, space=PSUM) as vp,
    ):
        for tt in range(8):  # 512-token tiles
            xb = xtp.tile([128, 8, 512], BF16, tag="xb", name="xb")
            nc.gpsimd.dma_start(xb[:], gath_x[128 * tt:128 * (tt + 1), :, :])
            for w, (wsb, bsb, dst) in enumerate(
                    ((wq_sb, bq_sb, QT), (wk_sb, bk_sb, KT))):
                acc = qkp.tile([128, 512], F32, tag=f"acc{w}",
                               name=f"acc{w}")
                for j in range(8):
                    nc.tensor.matmul(acc[:], wsb[:, j, :], xb[:, j, :],
                                     start=(j == 0), stop=False)
                nc.tensor.matmul(acc[:], bsb[:], ones_sb[:],
                                 start=False, stop=True)
                nc.scalar.activation(dst[:, 512 * tt:512 * (tt + 1)],
                                     acc[:], Identity, bias=0.0, scale=1.0)
            b_idx = tt // 4
            for tb in range(4):
                t0 = 128 * tb
                kc = (tt % 4) * 4 + tb
                vps = vp.tile([128, 128], F32, tag="vps", name="vps")
                for j in range(8):
                    nc.tensor.matmul(vps[:], xb[:, j, t0:t0 + 128],
                                     wv_sb[:, j, :],
                                     start=(j == 0), stop=False)
                nc.tensor.matmul(vps[:], ones_sb[:, 0:128], bv_sb[:],
                                 start=False, stop=True)
                for hh in range(2):
                    nc.scalar.activation(
                        Vn[2 * b_idx + hh][:, kc, 0:64],
                        vps[:, 64 * hh:64 * (hh + 1)],
                        Identity, bias=0.0, scale=1.0)

    # ---- Phase 2: attention per (batch, head) ----
    with (
        tc.tile_pool(name="pt", bufs=3) as ptp,
        tc.tile_pool(name="spsum", bufs=2, space=PSUM) as sp,
        tc.tile_pool(name="opsum", bufs=2, space=PSUM) as op,
        tc.tile_pool(name="bpsum", bufs=1, space=PSUM) as bp,
        tc.tile_pool(name="nrm", bufs=2) as nrm,
    ):
        for p in range(4):
            b_idx, hh = divmod(p, 2)
            base = 2048 * b_idx
            KT_h = KT[64 * hh:64 * (hh + 1), :]
            QT_h = QT[64 * hh:64 * (hh + 1), :]
            for qt in range(4):
                q0 = base + 512 * qt
                o_acc = op.tile([65, 512], F32, tag="oacc", name="o_acc")
                for kp in range(8):
                    s_ps = sp.tile([128, 1024], F32, tag="sps", name="s_ps")
                    for u in range(2):
                        k0 = base + 128 * (2 * kp + u)
                        nc.tensor.matmul(s_ps[:, 512 * u:512 * (u + 1)],
                                         KT_h[:, k0:k0 + 128],
                                         QT_h[:, q0:q0 + 512],
                                         start=True, stop=True)
                    pt_t = ptp.tile([128, 1024], F32R, tag="pt", name="pt_t")
                    nc.scalar.activation(pt_t[:], s_ps[:], Exp,
                                         bias=0.0, scale=0.125)
                    for u in range(2):
                        kc = 2 * kp + u
                        nc.tensor.matmul(o_acc[:], Vn[p][:, kc, :],
                                         pt_t[:, 512 * u:512 * (u + 1)],
                                         start=(kc == 0), stop=(kc == 15))
                r_f = nrm.tile([128, 512], F32, tag="rf", name="r_f")
                nc.vector.reciprocal(r_f[64:65, :], o_acc[64:65, :])
                r_t = nrm.tile([128, 512], F32R, tag="rt", name="r_t")
                nc.vector.tensor_copy(r_t[64:65, :], r_f[64:65, :])
                bc_ps = bp.tile([64, 512], F32, tag="bc", name="bc_ps")
                nc.tensor.matmul(bc_ps[:], onesr[64:65, :], r_t[64:65, :],
                                 start=True, stop=True)
                bc_sb = nrm.tile([64, 512], F32, tag="bcs", name="bc_sb")
                nc.vector.tensor_copy(bc_sb[:], bc_ps[:])
                nc.vector.tensor_mul(
                    OTb[64 * hh:64 * (hh + 1), q0:q0 + 512],
                    o_acc[0:64, :], bc_sb[:])

    # ---- Phase 3: partial out = OT^T @ wo_slice (+bo) ; ReduceScatter ----
    with (
        tc.tile_pool(name="fout", bufs=2) as fo,
        tc.tile_pool(name="fpsum", bufs=2, space=PSUM) as fp,
    ):
        for tb in range(32):
            t0 = 128 * tb
            acc = fp.tile([128, 1024], F32, tag="facc", name="acc")
            for ns in range(2):
                n0 = 512 * ns
                nc.tensor.matmul(acc[:, n0:n0 + 512], OTb[:, t0:t0 + 128],
                                 wo_sb[:, n0:n0 + 512],
                                 start=True, stop=False)
                nc.tensor.matmul(acc[:, n0:n0 + 512], ones_sb[:, 0:128],
                                 bo_sb[:, n0:n0 + 512],
                                 start=False, stop=True)
            o_sb = fo.tile([128, 1024], F32, tag="fo", name="o_sb")
            nc.scalar.activation(o_sb[:], acc[:], Identity,
                                 bias=0.0, scale=1.0)
            nc.gpsimd.dma_start(part_d[t0:t0 + 128, :], o_sb[:])
        nc.gpsimd.collective_compute(
            "ReduceScatter", mybir.AluOpType.add, replica_groups=groups,
            ins=[part_d[:].opt()], outs=[rs_d[:].opt()])
        nc.gpsimd.dma_start(out_d[:], rs_d[:])


def _in_maps(x, wq, bq, wk, bk, wv, bv, wo, bo):
    xt = x.reshape(TOK, D)

    def pack_x(c):
        xs = xt[512 * c:512 * (c + 1), :]
        return np.ascontiguousarray(
            xs.T.reshape(8, 128, 512).transpose(1, 0, 2)).astype(BF)

    def pack_w(w, c):
        wc = w[:, 128 * c:128 * (c + 1)]
        return np.ascontiguousarray(
            wc.reshape(8, 128, 128).transpose(1, 0, 2)).astype(BF)

    vones = np.ones((128, 16, 1), dtype=np.float32)
    onesb = np.ones((1, 512), dtype=np.float32).astype(BF)
    bo_row = bo.reshape(1, D).astype(BF)
    bo_zero = np.zeros((1, D), dtype=BF)
    maps = []
    for c in range(N_CORES):
        sl = slice(128 * c, 128 * (c + 1))
        maps.append({
            "xp": pack_x(c),
            "wqp": pack_w(wq, c), "wkp": pack_w(wk, c), "wvp": pack_w(wv, c),
            "wos": np.ascontiguousarray(wo[sl, :]).astype(BF),
            "bqr": bq[sl].reshape(1, 128).astype(BF),
            "bkr": bk[sl].reshape(1, 128).astype(BF),
            "bvr": bv[sl].reshape(1, 128).astype(BF),
            "bor": bo_row if c == 0 else bo_zero,
            "onesb": onesb,
            "vones": vones,
        })
    return maps


def kernel(**inputs):
    x = np.asarray(inputs["x"], dtype=np.float32)
    maps = _in_maps(
        x,
        np.asarray(inputs["wq"], np.float32), np.asarray(inputs["bq"], np.float32),
        np.asarray(inputs["wk"], np.float32), np.asarray(inputs["bk"], np.float32),
        np.asarray(inputs["wv"], np.float32), np.asarray(inputs["bv"], np.float32),
        np.asarray(inputs["wo"], np.float32), np.asarray(inputs["bo"], np.float32),
    )
    if "nc" not in _cache:
        _cache["nc"] = _build()
    res = run_bass_kernel_spmd(_cache["nc"], maps,
                               core_ids=list(range(N_CORES)), trace=False)
    out = np.concatenate([res.results[c]["out"] for c in range(N_CORES)],
                         axis=0)
    return out.reshape(B, S, D)


# revision 4
# speedup vs baseline: 2.3423x; 2.1485x over previous
"""MultiHeadAttention on 8 TRN2 NeuronCores — v3.

Like v2 but with minimal instruction count:
- Q/K/V all computed as [dk, tok] 8-chunk chains (N=512 moving) with the
  bias folded into the psum->SBUF ACT copy via a per-partition bias AP
  (no K=1 bias matmul chain elements).
- V transposed to natural [tok, dk] layout with 32 PE transposes
  ([128,128] each) + 32 single DVE scatter-copies into one 4D V tile
  (both heads per copy), replacing 256 narrow V matmuls + 64 ACT copies.
- Output-projection bias bo is added on the host after the gather
  (linear, so it commutes with ReduceScatter), dropping 64 K=1 matmuls.
"""

import numpy as np
import ml_dtypes

import concourse.bass as bass
import concourse.tile as tile
from concourse import bacc, mybir
from concourse.bass_utils import run_bass_kernel_spmd

N_CORES = 8
B, S, D = 2, 2048, 1024
TOK = B * S  # 4096
F32 = mybir.dt.float32
F32R = mybir.dt.float32r
BF16 = mybir.dt.bfloat16
Exp = mybir.ActivationFunctionType.Exp
Identity = mybir.ActivationFunctionType.Identity
BF = ml_dtypes.bfloat16

_cache = {}


def _build(repeat=1):
    nc = bacc.Bacc("TRN2", target_bir_lowering=False, debug=False,
                   num_devices=N_CORES)
    xp_d = nc.dram_tensor("xp", [128, 8, 512], BF16, kind="ExternalInput").ap()
    wq_d = nc.dram_tensor("wqp", [128, 8, 128], BF16, kind="ExternalInput").ap()
    wk_d = nc.dram_tensor("wkp", [128, 8, 128], BF16, kind="ExternalInput").ap()
    wv_d = nc.dram_tensor("wvp", [128, 8, 128], BF16, kind="ExternalInput").ap()
    wo_d = nc.dram_tensor("wos", [128, D], BF16, kind="ExternalInput").ap()
    bq_d = nc.dram_tensor("bqc", [128, 1], F32, kind="ExternalInput").ap()
    bk_d = nc.dram_tensor("bkc", [128, 1], F32, kind="ExternalInput").ap()
    bv_d = nc.dram_tensor("bvc", [128, 1], F32, kind="ExternalInput").ap()
    id_d = nc.dram_tensor("ident", [128, 128], F32, kind="ExternalInput").ap()
    vo_d = nc.dram_tensor("vones", [128, 16, 1], F32, kind="ExternalInput").ap()
    out_d = nc.dram_tensor("out", [512, D], F32, kind="ExternalOutput").ap()

    with tile.TileContext(nc) as tc:
        with (
            tc.tile_pool(name="dram", bufs=1, space="DRAM") as dram,
            tc.tile_pool(name="persist", bufs=1) as pp,
        ):
            xb_bo = dram.tile([128, 8, 512], BF16, tag="xbo")
            gath_x = dram.tile([1024, 8, 512], BF16, tag="gx")
            part_d = dram.tile([TOK, D], F32, tag="part")
            rs_d = dram.tile([512, D], F32, tag="rsd")

            wq_sb = pp.tile([128, 8, 128], BF16, tag="wq")
            wk_sb = pp.tile([128, 8, 128], BF16, tag="wk")
            wv_sb = pp.tile([128, 8, 128], BF16, tag="wv")
            wo_sb = pp.tile([128, D], BF16, tag="wo")
            nc.gpsimd.dma_start(wq_sb[:], wq_d[:])
            nc.gpsimd.dma_start(wk_sb[:], wk_d[:])
            nc.gpsimd.dma_start(wv_sb[:], wv_d[:])
            nc.gpsimd.dma_start(wo_sb[:], wo_d[:])
            bq_sb = pp.tile([128, 1], F32, tag="bq")
            bk_sb = pp.tile([128, 1], F32, tag="bk")
            bv_sb = pp.tile([128, 1], F32, tag="bv")
            id_sb = pp.tile([128, 128], F32R, tag="iden")
            nc.gpsimd.dma_start(bq_sb[:], bq_d[:])
            nc.gpsimd.dma_start(bk_sb[:], bk_d[:])
            nc.gpsimd.dma_start(bv_sb[:], bv_d[:])
            nc.gpsimd.dma_start(id_sb[:], id_d[:].bitcast(F32R))

            QT = pp.tile([128, TOK], F32R, tag="QT")
            KT = pp.tile([128, TOK], F32R, tag="KT")
            VT = pp.tile([128, TOK], F32R, tag="VT")
            Vbig = pp.tile([128, 4, 16, 65], F32R, tag="vbig")
            for p in range(4):
                nc.gpsimd.dma_start(Vbig[:, p, :, 64:65], vo_d[:].bitcast(F32R))
            OTb = pp.tile([128, TOK], BF16, tag="otb")

            ones_f = pp.tile([128, 64], F32, tag="onesf")
            nc.vector.memset(ones_f[:], 1.0)
            onesr = pp.tile([128, 64], F32R, tag="onesr")
            nc.vector.tensor_copy(onesr[:], ones_f[:])

            for _ in range(repeat):
                _body(nc, tc, xp_d, out_d, xb_bo, gath_x, part_d, rs_d,
                      (wq_sb, wk_sb, wv_sb), (bq_sb, bk_sb, bv_sb),
                      wo_sb, id_sb, onesr, QT, KT, VT, Vbig, OTb)
    nc.compile()
    return nc


def _body(nc, tc, xp_d, out_d, xb_bo, gath_x, part_d, rs_d,
          w_sbs, b_sbs, wo_sb, id_sb, onesr, QT, KT, VT, Vbig, OTb):
    PSUM = bass.MemorySpace.PSUM
    groups = [list(range(N_CORES))]
    wq_sb, wk_sb, wv_sb = w_sbs
    bq_sb, bk_sb, bv_sb = b_sbs

    # ---- AllGather x ----
    nc.gpsimd.dma_start(xb_bo[:], xp_d[:])
    nc.gpsimd.collective_compute(
        "AllGather", mybir.AluOpType.bypass, replica_groups=groups,
        ins=[xb_bo[:].opt()], outs=[gath_x[:].opt()])

    # ---- Phase 1: Q/K/V [dk, tok] chains; V transposed to [tok, dk] ----
    with (
        tc.tile_pool(name="xt", bufs=2) as xtp,
        tc.tile_pool(name="qkpsum", bufs=2, space=PSUM) as qkp,
        tc.tile_pool(name="tpsum", bufs=2, space=PSUM) as tpp,
    ):
        for tt in range(8):  # 512-token tiles
            xb = xtp.tile([128, 8, 512], BF16, tag="xb", name="xb")
            nc.gpsimd.dma_start(xb[:], gath_x[128 * tt:128 * (tt + 1), :, :])
            for w, (wsb, bsb, dst) in enumerate(
                    ((wq_sb, bq_sb, QT), (wk_sb, bk_sb, KT),
                     (wv_sb, bv_sb, VT))):
                acc = qkp.tile([128, 512], F32, tag=f"acc{w}",
                               name=f"acc{w}")
                for j in range(8):
                    nc.tensor.matmul(acc[:], wsb[:, j, :], xb[:, j, :],
                                     start=(j == 0), stop=(j == 7))
                nc.scalar.activation(dst[:, 512 * tt:512 * (tt + 1)],
                                     acc[:], Identity, bias=bsb[:], scale=1.0)
            b_idx = tt // 4
            for tb in range(4):
                t0 = 512 * tt + 128 * tb
                kc = (tt % 4) * 4 + tb
                tps = tpp.tile([128, 128], F32R, tag="tps", name="tps")
                nc.tensor.transpose(tps[:], VT[:, t0:t0 + 128], id_sb[:])
                nc.vector.tensor_copy(
                    Vbig[:, 2 * b_idx:2 * b_idx + 2, kc, 0:64], tps[:])

    # ---- Phase 2: attention per (batch, head) ----
    with (
        tc.tile_pool(name="pt", bufs=3) as ptp,
        tc.tile_pool(name="spsum", bufs=2, space=PSUM) as sp,
        tc.tile_pool(name="opsum", bufs=2, space=PSUM) as op,
        tc.tile_pool(name="bpsum", bufs=1, space=PSUM) as bp,
        tc.tile_pool(name="nrm", bufs=2) as nrm,
    ):
        for p in range(4):
            b_idx, hh = divmod(p, 2)
            base = 2048 * b_idx
            KT_h = KT[64 * hh:64 * (hh + 1), :]
            QT_h = QT[64 * hh:64 * (hh + 1), :]
            for qt in range(4):
                q0 = base + 512 * qt
                o_acc = op.tile([65, 512], F32, tag="oacc", name="o_acc")
                for kp in range(8):
                    s_ps = sp.tile([128, 1024], F32, tag="sps", name="s_ps")
                    for u in range(2):
                        k0 = base + 128 * (2 * kp + u)
                        nc.tensor.matmul(s_ps[:, 512 * u:512 * (u + 1)],
                                         KT_h[:, k0:k0 + 128],
                                         QT_h[:, q0:q0 + 512],
                                         start=True, stop=True)
                    pt_t = ptp.tile([128, 1024], F32R, tag="pt", name="pt_t")
                    nc.scalar.activation(pt_t[:], s_ps[:], Exp,
                                         bias=0.0, scale=0.125)
                    for u in range(2):
                        kc = 2 * kp + u
                        nc.tensor.matmul(o_acc[:], Vbig[:, p, kc, :],
                                         pt_t[:, 512 * u:512 * (u + 1)],
                                         start=(kc == 0), stop=(kc == 15))
                r_f = nrm.tile([128, 512], F32, tag="rf", name="r_f")
                nc.vector.reciprocal(r_f[64:65, :], o_acc[64:65, :])
                r_t = nrm.tile([128, 512], F32R, tag="rt", name="r_t")
                nc.vector.tensor_copy(r_t[64:65, :], r_f[64:65, :])
                bc_ps = bp.tile([64, 512], F32, tag="bc", name="bc_ps")
                nc.tensor.matmul(bc_ps[:], onesr[64:65, :], r_t[64:65, :],
                                 start=True, stop=True)
                bc_sb = nrm.tile([64, 512], F32, tag="bcs", name="bc_sb")
                nc.vector.tensor_copy(bc_sb[:], bc_ps[:])
                nc.vector.tensor_mul(
                    OTb[64 * hh:64 * (hh + 1), q0:q0 + 512],
                    o_acc[0:64, :], bc_sb[:])

    # ---- Phase 3: partial out = OT^T @ wo_slice ; ReduceScatter ----
    with (
        tc.tile_pool(name="fout", bufs=2) as fo,
        tc.tile_pool(name="fpsum", bufs=2, space=PSUM) as fp,
    ):
        for tb in range(32):
            t0 = 128 * tb
            acc = fp.tile([128, 1024], F32, tag="facc", name="acc")
            for ns in range(2):
                n0 = 512 * ns
                nc.tensor.matmul(acc[:, n0:n0 + 512], OTb[:, t0:t0 + 128],
                                 wo_sb[:, n0:n0 + 512],
                                 start=True, stop=True)
            o_sb = fo.tile([128, 1024], F32, tag="fo", name="o_sb")
            nc.scalar.activation(o_sb[:], acc[:], Identity,
                                 bias=0.0, scale=1.0)
            nc.gpsimd.dma_start(part_d[t0:t0 + 128, :], o_sb[:])
        nc.gpsimd.collective_compute(
            "ReduceScatter", mybir.AluOpType.add, replica_groups=groups,
            ins=[part_d[:].opt()], outs=[rs_d[:].opt()])
        nc.gpsimd.dma_start(out_d[:], rs_d[:])


def _in_maps(x, wq, bq, wk, bk, wv, bv, wo, bo):
    xt = x.reshape(TOK, D)

    def pack_x(c):
        xs = xt[512 * c:512 * (c + 1), :]
        return np.ascontiguousarray(
            xs.T.reshape(8, 128, 512).transpose(1, 0, 2)).astype(BF)

    def pack_w(w, c):
        wc = w[:, 128 * c:128 * (c + 1)]
        return np.ascontiguousarray(
            wc.reshape(8, 128, 128).transpose(1, 0, 2)).astype(BF)

    vones = np.ones((128, 16, 1), dtype=np.float32)
    ident = np.eye(128, dtype=np.float32)
    maps = []
    for c in range(N_CORES):
        sl = slice(128 * c, 128 * (c + 1))
        maps.append({
            "xp": pack_x(c),
            "wqp": pack_w(wq, c), "wkp": pack_w(wk, c), "wvp": pack_w(wv, c),
            "wos": np.ascontiguousarray(wo[sl, :]).astype(BF),
            "bqc": np.ascontiguousarray(bq[sl].reshape(128, 1)),
            "bkc": np.ascontiguousarray(bk[sl].reshape(128, 1)),
            "bvc": np.ascontiguousarray(bv[sl].reshape(128, 1)),
            "ident": ident,
            "vones": vones,
        })
    return maps


def kernel(**inputs):
    x = np.asarray(inputs["x"], dtype=np.float32)
    bo = np.asarray(inputs["bo"], np.float32)
    maps = _in_maps(
        x,
        np.asarray(inputs["wq"], np.float32), np.asarray(inputs["bq"], np.float32),
        np.asarray(inputs["wk"], np.float32), np.asarray(inputs["bk"], np.float32),
        np.asarray(inputs["wv"], np.float32), np.asarray(inputs["bv"], np.float32),
        np.asarray(inputs["wo"], np.float32), bo,
    )
    if "nc" not in _cache:
        _cache["nc"] = _build()
    res = run_bass_kernel_spmd(_cache["nc"], maps,
                               core_ids=list(range(N_CORES)), trace=False)
    out = np.concatenate([res.results[c]["out"] for c in range(N_CORES)],
                         axis=0)
    return (out + bo.reshape(1, D)).reshape(B, S, D)
